# revision 35
# baseline (speedup 1.0000x reference)
"""SS3D (3D selective scan / VMamba block) Trainium2 kernel, 8-core SPMD.

Sharding (core-uniform program, all per-core variation rides on input data):
  scan-role(core c) = (b, dh, nh): b = batch (c>>2), dh = d_inner half
  ((c>>1)&1, 128 of 256), nh = state half (c&1, 8 of 16).  All 12 scan
  directions run on every core for its (b, dh, nh) slice; direction geometry
  is static APs (same on every core).
Key algorithm facts:
  - A = -exp(A_logs) per (k,d,n) enters only as dA = exp(A * delta) -> one
    Exp activation per n with per-partition scale column (exact for any A).
  - directions k>=6 are flips: handled entirely by negated-stride APs; the
    scan itself always runs forward.
  - sum_k Ds_k * invperm(xs_k) = (sum_k Ds_k) * xc  (Ds fold, one pass).
I/O design (warm calls are axon-tunnel latency bound, ~90ms RTT, ~50MB/s):
  - per-call traffic is x up as int8 + per-token scales (dequantized on
    device; sharded token rows, no host rearrangement) and out down as int8 +
    per-token scales (quantized on device after the final transpose); x is
    transposed and allgathered on device, conv runs full-channel per core for
    its batch.  Both d2h fetches are issued async so they overlap.
  - weights are device-resident jax arrays, re-uploaded only when their
    content fingerprint changes; the jitted executable is built once, with
    both donation specializations pre-warmed.
  - output buffers from call N are donated back as the exec scratch of call
    N+1 (the kernel overwrites every element of both outputs).
"""
import hashlib

import numpy as np
import ml_dtypes

import concourse.bass as bass
import concourse.tile as tile
from concourse import bacc, mybir

N_CORES = 8
GP_FRAC = 0
NW_BUFS = 8
F32, BF16, I32 = mybir.dt.float32, mybir.dt.bfloat16, mybir.dt.int32
AF = mybir.ActivationFunctionType
OP = mybir.AluOpType
BF16NP = ml_dtypes.bfloat16

B, Dd, H, W = 2, 16, 16, 16
L = Dd * H * W               # 4096
DM, DN, NST, RK = 128, 256, 16, 8
K = 12
ORDERS = [(2, 3, 4), (2, 4, 3), (3, 2, 4), (3, 4, 2), (4, 2, 3), (4, 3, 2)]
SSTR = (256, 16, 1)          # strides of (z,y,x) in flat l
NCH = 8                      # 512-col chunks per L
CH = 512


def _sap(t_ap, off, dims):
    return bass.AP(t_ap.tensor, t_ap.offset + off,
                   [list(t_ap.ap[0])] + [list(d) for d in dims])


def _perm_dims(k, chunks=None, chunk_idx=0):
    """Free-dim [step,count] triple + offset for direction k (flip if k>=6)."""
    o = ORDERS[k % 6]
    p = [oo - 2 for oo in o]
    s = [SSTR[p[0]], SSTR[p[1]], SSTR[p[2]]]
    if k >= 6:
        off = 4095
        s = [-x for x in s]
    else:
        off = 0
    dims = [[s[0], 16], [s[1], 16], [s[2], 16]]
    if chunks is not None:
        n_out = 16 // chunks
        dims = [[s[0], n_out], [s[1], 16], [s[2], 16]]
        off = off + chunk_idx * n_out * s[0]
    return off, dims


def _patch_act_tables():
    # The greedy table chooser assigns Exp->exp_and_others and Ln->natural_log,
    # reloading ACT tables on every softplus (128 loads/kernel).  Restrict the
    # choosable tables (keeping act_func_set_id positions) so Exp+Ln+Copy all
    # resolve inside natural_log_exp_and_others.
    import concourse.bacc as _bm
    if getattr(_bm, "_act_tables_patched", False):
        return
    _orig = _bm.get_activation_tables
    _keep = {"natural_log_exp_and_others", "silu_and_others", "sqrt_and_others"}
    def _patched(arch):
        t = _orig(arch)
        return {k: (v if k in _keep else set()) for k, v in t.items()}
    _bm.get_activation_tables = _patched
    _bm._act_tables_patched = True


def _build(sim=False):
    _patch_act_tables()
    nc = bacc.Bacc(None, target_bir_lowering=False, debug=False, num_devices=N_CORES)

    def din(name, shape, dt=F32):
        return nc.dram_tensor(name, shape, dt, kind="ExternalInput").ap()

    I8 = mybir.dt.int8
    # --- inputs ---
    x_tok = din("x_tok", [1024, 128], I8)             # my token rows of x (int8/row)
    x_scale = din("x_scale", [1024, 1])               # per-token dequant scales
    w_inproj = din("w_inproj", [128, 256], BF16)      # in_proj conv lhsT, full 256 ch
    w_conv = din("w_conv", [128, 54 * 128], BF16)     # 2 ch-tiles x 27 diag(w_tap) lhsT
    b_conv = din("b_conv", [128, 2])                  # col t = conv_b for ch-tile t
    w_z = din("w_z", [128, 256], BF16)                # z-gate in_proj lhsT
    w_xproj = din("w_xproj", [128, K * 2 * 24], BF16) # lhsT chunks per k (nh-selected)
    w_dt = din("w_dt", [8, K * 128], BF16)            # lhsT per k (dh slice)
    b_dt = din("b_dt", [128, K])
    a_scale = din("a_scale", [128, K * 8])            # per-partition Exp scales
    ds_sum = din("ds_sum", [128, 1])
    w_out = din("w_out", [128, 256], BF16)
    g_rep = din("g_rep", [128, 256])
    br_rep = din("b_rep", [128, 256])
    ident = din("ident", [128, 128], BF16)
    eps_in = din("eps_in", [128, 1])
    idx_xc = din("idx_xc", [128, 1], I32)             # xcd rows of my d-half
    idx_y = din("idx_y", [128, 4], I32)
    out8 = nc.dram_tensor("out8", [1024, 128], I8, kind="ExternalOutput").ap()
    out_s = nc.dram_tensor("out_s", [1024, 1], F32, kind="ExternalOutput").ap()

    with tile.TileContext(nc) as tc:
        with (
            tc.tile_pool(name="const", bufs=1) as cp,
            tc.tile_pool(name="big", bufs=1) as bigp,
            tc.tile_pool(name="kwork", bufs=1) as kp,
            tc.tile_pool(name="nwork", bufs=NW_BUFS) as nw,
            tc.tile_pool(name="small", bufs=2) as sm,
            tc.tile_pool(name="pers", bufs=1) as pr,
            tc.tile_pool(name="ps", bufs=2, space="PSUM") as ps,
            tc.tile_pool(name="ps2", bufs=1, space="PSUM") as ps2,
            tc.tile_pool(name="dram", bufs=1, space="DRAM") as dp,
        ):
            def load(ap_in, shape, dt=F32, pool=cp):
                nm = ap_in.name + "_sb"
                t = pool.tile(shape, dt, name=nm, tag=nm)
                nc.sync.dma_start(t[:], ap_in[:])
                return t

            wim = load(w_inproj, [128, 256], BF16)
            wcv = load(w_conv, [128, 54 * 128], BF16)
            bcv = load(b_conv, [128, 2])
            wz = load(w_z, [128, 256], BF16)
            wxp = load(w_xproj, [128, K * 2 * 24], BF16)
            wdt = load(w_dt, [8, K * 128], BF16)
            bdt = load(b_dt, [128, K])
            asc = load(a_scale, [128, K * 8])
            dss = load(ds_sum, [128, 1])
            wo = load(w_out, [128, 256], BF16)
            gr = load(g_rep, [128, 256])
            br = load(br_rep, [128, 256])
            idn = load(ident, [128, 128], BF16)
            epsv = load(eps_in, [128, 1])
            ixc = load(idx_xc, [128, 1], I32)
            iy = load(idx_y, [128, 4], I32)

            # ---------------- Stage A: load my tokens (int8), dequant, transpose
            xsc = cp.tile([128, 8], F32, tag="xsc")
            nc.sync.dma_start(xsc[:], bass.AP(x_scale.tensor, 0, [[1, 128], [128, 8]]))
            xtk8 = bigp.tile([128, 1024], I8, tag="xtk8")
            nc.sync.dma_start(
                _sap(xtk8[:], 0, [[128, 8], [1, 128]]),
                bass.AP(x_tok.tensor, 0, [[128, 128], [16384, 8], [1, 128]]))
            xtk = bigp.tile([128, 1024], BF16, tag="xtk")
            for j in range(8):
                nc.scalar.activation(xtk[:, j * 128:(j + 1) * 128],
                                     xtk8[:, j * 128:(j + 1) * 128], AF.Copy,
                                     scale=xsc[:, j:j + 1])
            xtT = bigp.tile([128, 1024], BF16, tag="xtT")
            for j in range(8):
                tp = ps.tile([128, 128], BF16, tag="ps_a", name="tp_x")
                nc.tensor.transpose(tp[:], xtk[:, j * 128:(j + 1) * 128], idn[:])
                nc.scalar.copy(xtT[:, j * 128:(j + 1) * 128], tp[:])

            # ---------------- Stage B: allgather token columns (batch group of 4:
            # a core only needs its batch's tokens, and batch b lives exactly on
            # cores 4b..4b+3 -> group rows are statically addressable)
            ag_in = dp.tile([128, 1024], BF16)
            xg = dp.tile([512, 1024], BF16)
            nc.gpsimd.dma_start(ag_in[:], xtT[:])
            if sim:
                for _q in range(4):
                    nc.gpsimd.dma_start(xg[_q * 128:(_q + 1) * 128, :], ag_in[:])
            else:
                nc.gpsimd.collective_compute(
                    "AllGather", OP.bypass,
                    replica_groups=[[0, 1, 2, 3], [4, 5, 6, 7]],
                    ins=[ag_in.opt()], outs=[xg.opt()])

            # ---------------- Stage B2: z-gate from local tokens (overlaps allgather)
            zg = []
            for tch in range(2):
                zt = pr.tile([128, 1024], BF16, tag=f"zg{tch}", name=f"zg{tch}")
                for c2 in range(2):
                    zp = ps.tile([128, 512], F32, tag="ps_b", name="zp")
                    nc.tensor.matmul(zp[:], wz[:, tch * 128:(tch + 1) * 128],
                                     xtT[:, c2 * 512:(c2 + 1) * 512], start=True, stop=True)
                    nc.scalar.activation(zt[:, c2 * 512:(c2 + 1) * 512], zp[:], AF.Silu)
                zg.append(zt)

            # ---------------- Stage C: my batch's x_t slab (static group rows)
            xcv = bigp.tile([128, 4096], BF16, tag="xcv")
            for q in range(4):
                nc.gpsimd.dma_start(xcv[:, q * 1024:(q + 1) * 1024],
                                    xg[q * 128:(q + 1) * 128, :])

            # ---------------- Stage D: in_proj + depthwise conv + silu (full 256 ch)
            pads = []
            for t in range(2):
                pad = bigp.tile([128, 18 * 324], BF16, tag=f"pad{t}", name=f"pad{t}")
                nc.gpsimd.memset(pad[:], 0.0)
                pads.append(pad)
            for t in range(2):
                for c in range(8):
                    mp = ps.tile([128, 512], F32, tag="ps_a", name="mp")
                    nc.tensor.matmul(mp[:], wim[:, t * 128:(t + 1) * 128],
                                     xcv[:, c * 512:(c + 1) * 512], start=True, stop=True)
                    dst = _sap(pads[t][:], (1 + 2 * c) * 324 + 19,
                               [[324, 2], [18, 16], [1, 16]])
                    src3 = _sap(mp[:], 0, [[256, 2], [16, 16], [1, 16]])
                    nc.scalar.activation(dst, src3, AF.Copy)
            xc_b = []
            for t in range(2):
                # t=0 reuses the xcv slot (dead after the in_proj drains above)
                xcb = bigp.tile([128, 4096], BF16, tag=("xcv" if t == 0 else "xcb1"),
                                name=f"xcb{t}")
                for c in range(8):
                    cps = ps.tile([128, 512], F32, tag="ps_a", name="cps")
                    tap = 0
                    for dz in range(3):
                        for dy in range(3):
                            for dx in range(3):
                                src = _sap(pads[t][:], (2 * c + dz) * 324 + dy * 18 + dx,
                                           [[324, 2], [18, 16], [1, 16]])
                                nc.tensor.matmul(cps[:], wcv[:, (t * 27 + tap) * 128:(t * 27 + tap + 1) * 128],
                                                 src, start=(tap == 0), stop=(tap == 26))
                                tap += 1
                    nc.scalar.activation(xcb[:, c * 512:(c + 1) * 512], cps[:],
                                         AF.Silu, bias=bcv[:, t:t + 1])
                xc_b.append(xcb)

            # ---------------- Stage E: select my d-half via DRAM round trip
            xcd = dp.tile([256, 4096], BF16)
            for t in range(2):
                nc.gpsimd.dma_start(xcd[t * 128:(t + 1) * 128, :], xc_b[t][:])
            xc_my = bigp.tile([128, 4096], BF16, tag="xc_my")
            nc.gpsimd.indirect_dma_start(
                out=xc_my[:], out_offset=None, in_=xcd[:],
                in_offset=bass.IndirectOffsetOnAxis(ap=ixc[:, 0:1], axis=0))

            # ---------------- Stage F: 12 directions
            ycum = bigp.tile([128, 4096], F32, tag="pad0", name="ycum")
            # Ds fold: ycum = xc_my * ds_sum
            nc.vector.tensor_scalar(ycum[:], xc_my[:], dss[:, 0:1], None, OP.mult)

            mulidx = 0
            for k in range(K):
                # x_proj with perm applied at the matmul rhs; combined bf16 drain:
                # pkb rows = [dtr(8); B_my(8); C_my(8)] in direction-k scan order
                pkb = kp.tile([24, 4096], BF16, tag="pkb")
                for c in range(NCH):
                    off, dims = _perm_dims(k, chunks=NCH, chunk_idx=c)
                    pp = ps.tile([24, 512], F32, tag="ps_a", name="pp")
                    for tch in range(2):
                        nc.tensor.matmul(
                            pp[:], wxp[:, k * 48 + tch * 24: k * 48 + (tch + 1) * 24],
                            _sap(xc_b[tch][:], off, dims),
                            start=(tch == 0), stop=(tch == 1))
                    nc.scalar.copy(pkb[:, c * CH:(c + 1) * CH], pp[:])
                # stage B/C rows in DRAM for broadcast-read DMAs
                psig_d = dp.tile([16, 4096], BF16, tag="psig_d", name="psig_d", bufs=2)
                nc.sync.dma_start(psig_d[:], pkb[8:24, :])
                # dts -> delta = softplus = ln(1 + exp(.)): Exp per chunk (PSUM src),
                # Ln as one full-length pass
                delta = kp.tile([128, 4096], BF16, tag="delta")
                et = kp.tile([128, 4096], BF16, tag="et")
                for c in range(NCH):
                    dp_ = ps.tile([128, 512], F32, tag="ps_b", name="dp_")
                    nc.tensor.matmul(dp_[:], wdt[:, k * 128:(k + 1) * 128],
                                     pkb[0:8, c * CH:(c + 1) * CH], start=True, stop=True)
                    nc.scalar.activation(et[:, c * CH:(c + 1) * CH], dp_[:], AF.Exp,
                                         bias=bdt[:, k:k + 1])
                nc.scalar.activation(delta[:], et[:], AF.Ln, bias=1.0)
                # xs = perm-strided copy of xc_my (ACT handles 4D APs)
                xs = kp.tile([128, 4096], BF16, tag="et", name="xs")
                off, dims = _perm_dims(k)
                d3 = [[256, 16], [16, 16], [1, 16]]
                nc.scalar.activation(_sap(xs[:], 0, d3), _sap(xc_my[:], off, dims), AF.Copy)
                du = kp.tile([128, 4096], BF16, tag="du")
                nc.vector.tensor_tensor(out=du[:], in0=delta[:], in1=xs[:], op=OP.mult)
                hcol = kp.tile([128, 8], F32, tag="hcol")
                for half in range(2):
                    hs = slice(half * 2048, (half + 1) * 2048)
                    yk_ps = ps2.tile([128, 2048], F32, tag="yk_ps")
                    for n in range(8):
                        dA = nw.tile([128, 2048], BF16, tag="nw1", name="dA")
                        nc.scalar.activation(dA[:], delta[:, hs], AF.Exp,
                                             scale=asc[:, k * 8 + n:k * 8 + n + 1])
                        brep = nw.tile([128, 2048], BF16, tag="nw1", name="brep")
                        nc.sync.dma_start(brep[:], bass.AP(psig_d[:].tensor,
                                          psig_d[:].offset + n * 4096 + half * 2048,
                                          [[0, 128], [1, 2048]]))
                        crep = nw.tile([128, 2048], BF16, tag="nw1", name="crep")
                        nc.scalar.dma_start(crep[:], bass.AP(psig_d[:].tensor,
                                            psig_d[:].offset + (8 + n) * 4096 + half * 2048,
                                            [[0, 128], [1, 2048]]))
                        dBu = nw.tile([128, 2048], BF16, tag="dBu")
                        eng1 = nc.gpsimd if (mulidx % 12) < GP_FRAC else nc.vector
                        mulidx += 1
                        eng1.tensor_tensor(out=dBu[:], in0=du[:, hs], in1=brep[:], op=OP.mult)
                        init = 0.0 if half == 0 else hcol[:, n:n + 1]
                        nc.vector.tensor_tensor_scan(dBu[:], dA[:], dBu[:], init,
                                                     OP.mult, OP.add)
                        h = dBu
                        if half == 0:
                            nc.vector.tensor_copy(hcol[:, n:n + 1], h[:, 2047:2048])
                        eng2 = nc.gpsimd if (mulidx % 12) < GP_FRAC else nc.vector
                        mulidx += 1
                        eng2.tensor_tensor(out=h[:], in0=h[:], in1=crep[:], op=OP.mult)
                        for c4 in range(4):
                            nc.tensor.matmul(yk_ps[:, c4 * 512:(c4 + 1) * 512], idn[:],
                                             h[:, c4 * 512:(c4 + 1) * 512],
                                             start=(n == 0), stop=(n == 7))
                    # accumulate this half into ycum at inverse-permuted positions
                    off, dims = _perm_dims(k, chunks=2, chunk_idx=half)
                    dst = _sap(ycum[:], off, dims)
                    nc.vector.tensor_tensor(out=dst, in0=dst,
                                            in1=_sap(yk_ps[:], 0, [[256, 8], [16, 16], [1, 16]]),
                                            op=OP.add)

            # ---------------- collective: allgather y quadrants (batch group of 4)
            yg_in = dp.tile([128, 4096], BF16)
            yg_out = dp.tile([512, 4096], BF16)
            nc.gpsimd.dma_start(yg_in[:], ycum[:])
            if sim:
                for _q in range(4):
                    nc.gpsimd.dma_start(yg_out[_q * 128:(_q + 1) * 128, :], yg_in[:])
            else:
                nc.gpsimd.collective_compute(
                    "AllGather", OP.bypass,
                    replica_groups=[[0, 1, 2, 3], [4, 5, 6, 7]],
                    ins=[yg_in.opt()], outs=[yg_out.opt()])

            # ---------------- post: my 1024 tokens
            ygv = bass.AP(yg_out[:].tensor, 0, [[1024, 2048], [1, 1024]])  # (2048,1024) view
            yhalf = []
            for dhp in range(2):
                ta = pr.tile([128, 1024], BF16, tag=f"ya{dhp}", name=f"ya{dhp}")
                tb = sm.tile([128, 1024], BF16, tag="yb")
                nc.gpsimd.indirect_dma_start(
                    out=ta[:], out_offset=None, in_=ygv,
                    in_offset=bass.IndirectOffsetOnAxis(ap=iy[:, 2 * dhp:2 * dhp + 1], axis=0))
                nc.gpsimd.indirect_dma_start(
                    out=tb[:], out_offset=None, in_=ygv,
                    in_offset=bass.IndirectOffsetOnAxis(ap=iy[:, 2 * dhp + 1:2 * dhp + 2], axis=0))
                nc.vector.tensor_tensor(out=ta[:], in0=ta[:], in1=tb[:], op=OP.add)
                yhalf.append(ta)

            ynT = [pr.tile([128, 1024], BF16, tag="ynT0", name="ynT0"),
                   pr.tile([128, 1024], BF16, tag="ynT1", name="ynT1")]
            for j in range(8):    # token blocks of 128
                yT = sm.tile([128, 256], F32, tag="yT")
                for dhp in range(2):
                    tp = ps.tile([128, 128], BF16, tag="ps_a", name="tp_y")
                    nc.tensor.transpose(tp[:], yhalf[dhp][:, j * 128:(j + 1) * 128], idn[:])
                    nc.scalar.copy(yT[:, dhp * 128:(dhp + 1) * 128], tp[:])
                # LayerNorm over 256 channels (free dim)
                nmu = sm.tile([128, 1], F32, tag="nmu")
                nc.vector.tensor_reduce(nmu[:], yT[:], mybir.AxisListType.X, OP.add, negate=True)
                nc.scalar.mul(nmu[:], nmu[:], 1.0 / 256)
                sq = sm.tile([128, 256], F32, tag="sq")
                nc.scalar.activation(sq[:], yT[:], AF.Square)
                ssq = sm.tile([128, 1], F32, tag="ssq")
                nc.vector.tensor_reduce(ssq[:], sq[:], mybir.AxisListType.X, OP.add)
                musq = sm.tile([128, 1], F32, tag="musq")
                nc.scalar.activation(musq[:], nmu[:], AF.Square)
                var = sm.tile([128, 1], F32, tag="var")
                nc.vector.scalar_tensor_tensor(var[:], ssq[:], 1.0 / 256, musq[:],
                                               OP.mult, OP.subtract)
                std = sm.tile([128, 1], F32, tag="std")
                nc.scalar.activation(std[:], var[:], AF.Sqrt, bias=epsv[:, 0:1])
                inv = sm.tile([128, 1], F32, tag="inv")
                nc.vector.reciprocal(inv[:], std[:])
                bmu = sm.tile([128, 1], F32, tag="bmu")
                nc.vector.tensor_tensor(out=bmu[:], in0=nmu[:], in1=inv[:], op=OP.mult)
                yn = sm.tile([128, 256], BF16, tag="yn")
                nc.scalar.activation(yn[:], yT[:], AF.Identity, bias=bmu[:, 0:1], scale=inv[:, 0:1])
                nc.vector.tensor_tensor(out=yn[:], in0=yn[:], in1=gr[:], op=OP.mult)
                nc.vector.tensor_tensor(out=yn[:], in0=yn[:], in1=br[:], op=OP.add)
                for dhp in range(2):
                    tp = ps.tile([128, 128], BF16, tag="ps_b", name="tp_n")
                    nc.tensor.transpose(tp[:], yn[:, dhp * 128:(dhp + 1) * 128], idn[:])
                    nc.scalar.copy(ynT[dhp][:, j * 128:(j + 1) * 128], tp[:])
            # gate + out_proj, then transpose to token-major bf16 out
            for tch in range(2):
                nc.vector.tensor_tensor(out=ynT[tch][:], in0=ynT[tch][:], in1=zg[tch][:],
                                        op=OP.mult)
            for c2 in range(2):
                op_ = ps.tile([128, 512], F32, tag="ps_a", name="op_")
                for tch in range(2):
                    nc.tensor.matmul(op_[:], wo[:, tch * 128:(tch + 1) * 128],
                                     ynT[tch][:, c2 * 512:(c2 + 1) * 512],
                                     start=(tch == 0), stop=(tch == 1))
                ost = sm.tile([128, 512], BF16, tag="osb", name="osb")
                nc.scalar.copy(ost[:], op_[:])
                for j4 in range(4):
                    tp = ps.tile([128, 128], BF16, tag="ps_b", name="tp_o")
                    nc.tensor.transpose(tp[:], ost[:, j4 * 128:(j4 + 1) * 128], idn[:])
                    ob = sm.tile([128, 128], BF16, tag="ob", name="ob")
                    nc.scalar.copy(ob[:], tp[:])
                    # int8/row quantize: partition = token after the transpose
                    sqo = sm.tile([128, 128], F32, tag="sqo", name="sqo")
                    nc.scalar.activation(sqo[:], ob[:], AF.Square)
                    mxs = sm.tile([128, 1], F32, tag="mxs", name="mxs")
                    nc.vector.tensor_reduce(mxs[:], sqo[:], mybir.AxisListType.X, OP.max)
                    amo = sm.tile([128, 1], F32, tag="amo", name="amo")
                    nc.scalar.activation(amo[:], mxs[:], AF.Sqrt)
                    qsc = sm.tile([128, 1], F32, tag="qsc", name="qsc")
                    nc.vector.reciprocal(qsc[:], amo[:])
                    nc.scalar.mul(qsc[:], qsc[:], 127.0)
                    osc = sm.tile([128, 1], F32, tag="osc", name="osc")
                    nc.scalar.mul(osc[:], amo[:], 1.0 / 127.0)
                    ob8 = sm.tile([128, 128], I8, tag="ob8", name="ob8")
                    nc.scalar.activation(ob8[:], ob[:], AF.Copy, scale=qsc[:, 0:1])
                    r0 = (c2 * 4 + j4) * 128
                    nc.sync.dma_start(out8[r0:r0 + 128, :], ob8[:])
                    nc.sync.dma_start(out_s[r0:r0 + 128, :], osc[:])

    nc.compile()
    return nc


def _prep_weights(inputs):
    """name -> global (8*rows, ...) array, in the kernel's input layouts."""
    in_proj_w = np.asarray(inputs["in_proj_w"], np.float32)
    conv_w = np.asarray(inputs["conv_w"], np.float32).reshape(DN, 27)
    conv_b = np.asarray(inputs["conv_b"], np.float32)
    x_proj_weight = np.asarray(inputs["x_proj_weight"], np.float32)
    dt_projs_weight = np.asarray(inputs["dt_projs_weight"], np.float32)
    dt_projs_bias = np.asarray(inputs["dt_projs_bias"], np.float32).reshape(K, DN)
    A = -np.exp(np.asarray(inputs["A_logs"], np.float32)).reshape(K, DN, NST)
    Ds = np.asarray(inputs["Ds"], np.float32).reshape(K, DN)
    out_norm_g = np.asarray(inputs["out_norm_g"], np.float32)
    out_norm_b = np.asarray(inputs["out_norm_b"], np.float32)
    out_proj_w = np.asarray(inputs["out_proj_w"], np.float32)
    ds_sum_all = Ds.sum(0)

    sh = {}
    sh["w_inproj"] = in_proj_w[0:DN, :].T.astype(BF16NP)
    wcv = np.zeros((128, 54 * 128), np.float32)
    i = np.arange(128)
    for t in range(2):
        for tap in range(27):
            wcv[i, (t * 27 + tap) * 128 + i] = conv_w[t * 128 + i, tap]
    sh["w_conv"] = wcv.astype(BF16NP)
    sh["b_conv"] = conv_b.reshape(2, 128).T.copy()
    sh["w_z"] = in_proj_w[DN:2 * DN, :].T.astype(BF16NP)
    wo = np.zeros((128, 256), np.float32)
    for tch in range(2):
        wo[:, tch * 128:(tch + 1) * 128] = out_proj_w[:, tch * 128:(tch + 1) * 128].T
    sh["w_out"] = wo.astype(BF16NP)
    sh["g_rep"] = np.tile(out_norm_g[None, :], (128, 1)).astype(np.float32)
    sh["b_rep"] = np.tile(out_norm_b[None, :], (128, 1)).astype(np.float32)
    sh["ident"] = np.eye(128, dtype=BF16NP)
    sh["eps_in"] = np.full((128, 1), 1e-5, np.float32)

    per = {nm: [] for nm in ["w_xproj", "w_dt", "b_dt", "a_scale", "ds_sum",
                             "idx_xc", "idx_y"]}
    for c in range(N_CORES):
        b, dh, nh = c >> 2, (c >> 1) & 1, c & 1
        dsl = slice(dh * 128, dh * 128 + 128)
        wxp = np.zeros((128, K * 2 * 24), np.float32)
        rows = list(range(8)) + list(range(8 + 8 * nh, 16 + 8 * nh)) + \
               list(range(24 + 8 * nh, 32 + 8 * nh))
        for k in range(K):
            Wsel = x_proj_weight[k][rows]                     # (24, 256)
            for tch in range(2):
                wxp[:, k * 48 + tch * 24: k * 48 + (tch + 1) * 24] = \
                    Wsel[:, tch * 128:(tch + 1) * 128].T
        per["w_xproj"].append(wxp.astype(BF16NP))
        wdt = np.zeros((8, K * 128), np.float32)
        for k in range(K):
            wdt[:, k * 128:(k + 1) * 128] = dt_projs_weight[k][dsl].T
        per["w_dt"].append(wdt.astype(BF16NP))
        per["b_dt"].append(np.ascontiguousarray(dt_projs_bias[:, dsl].T))
        asc = np.zeros((128, K * 8), np.float32)
        for k in range(K):
            asc[:, k * 8:(k + 1) * 8] = A[k, dsl, nh * 8:nh * 8 + 8]
        per["a_scale"].append(asc)
        per["ds_sum"].append(np.ascontiguousarray(ds_sum_all[dsl, None]))
        per["idx_xc"].append((dh * 128 + np.arange(128)).astype(np.int32)[:, None])
        iy = np.zeros((128, 4), np.int32)
        tokblock = c & 3
        for dhp in range(2):
            for nhp in range(2):
                q = (dhp << 1) | nhp                  # group-local quadrant
                iy[:, 2 * dhp + nhp] = (q * 128 + np.arange(128)) * 4 + tokblock
        per["idx_y"].append(iy)

    g = {nm: np.concatenate([arr] * N_CORES, axis=0) for nm, arr in sh.items()}
    for nm, lst in per.items():
        g[nm] = np.concatenate(lst, axis=0)
    return g


_WKEYS = ["in_proj_w", "conv_w", "conv_b", "x_proj_weight", "dt_projs_weight",
          "dt_projs_bias", "A_logs", "Ds", "out_norm_g", "out_norm_b", "out_proj_w"]


def _weight_digest(inputs):
    h = hashlib.blake2b(digest_size=16)
    for kk in _WKEYS:
        a = np.ascontiguousarray(np.asarray(inputs[kk]))
        h.update(a.tobytes())
    return h.hexdigest()


_ST = None


def _get_state():
    global _ST
    if _ST is not None:
        return _ST
    import jax
    from jax.sharding import Mesh, PartitionSpec, NamedSharding
    from jax.experimental.shard_map import shard_map
    from concourse.bass2jax import (_bass_exec_p, partition_id_tensor,
                                    install_neuronx_cc_hook)

    nc = _build()
    install_neuronx_cc_hook()

    partition_name = nc.partition_id_tensor.name if nc.partition_id_tensor else None
    in_names, out_names, out_avals, zero_outs, in_specs_np = [], [], [], [], {}
    for alloc in nc.m.functions[0].allocations:
        if not isinstance(alloc, mybir.MemoryLocationSet):
            continue
        name = alloc.memorylocations[0].name
        if alloc.kind == "ExternalInput":
            if name != partition_name:
                in_names.append(name)
                in_specs_np[name] = (tuple(alloc.tensor_shape),
                                     mybir.dt.np(alloc.dtype))
        elif alloc.kind == "ExternalOutput":
            out_names.append(name)
            shape = tuple(alloc.tensor_shape)
            dtype = mybir.dt.np(alloc.dtype)
            out_avals.append(jax.core.ShapedArray(shape, dtype))
            zero_outs.append(np.zeros((N_CORES * shape[0], *shape[1:]), dtype))
    n_params = len(in_names)
    n_outs = len(out_avals)
    all_in = list(in_names) + list(out_names)
    if partition_name is not None:
        all_in.append(partition_name)

    def _body(*args):
        operands = list(args)
        if partition_name is not None:
            operands.append(partition_id_tensor())
        outs = _bass_exec_p.bind(
            *operands, out_avals=tuple(out_avals), in_names=tuple(all_in),
            out_names=tuple(out_names), lowering_input_output_aliases=(),
            sim_require_finite=True, sim_require_nnan=True, nc=nc)
        return tuple(outs)

    devices = jax.devices()[:N_CORES]
    mesh = Mesh(np.asarray(devices), ("core",))
    in_specs = (PartitionSpec("core"),) * (n_params + n_outs)
    out_specs = (PartitionSpec("core"),) * n_outs
    donate = tuple(range(n_params, n_params + n_outs))
    fn = jax.jit(shard_map(_body, mesh=mesh, in_specs=in_specs,
                           out_specs=out_specs, check_rep=False),
                 donate_argnums=donate, keep_unused=True)
    shard = NamedSharding(mesh, PartitionSpec("core"))
    # Pre-warm both jit specializations (call 1: numpy donated zeros; call 2+:
    # committed device weights + donated device outputs), so the first two real
    # calls don't pay ~0.5s of XLA re-specialization each.
    dzero = {nm: np.zeros((N_CORES * s[0], *s[1:]), dt)
             for nm, (s, dt) in in_specs_np.items()}
    dummy_w = {nm: jax.device_put(dzero[nm], shard) for nm in in_names}
    args_w = [dummy_w[nm] for nm in in_names]
    outs = fn(*args_w, *zero_outs)      # spec of real call 1 (numpy donated zeros)
    outs = fn(*args_w, *outs)           # spec of calls 2+ (donated device outputs)
    jax.block_until_ready(outs)
    hostbuf = dict(xqf=np.empty((N_CORES * 1024, 128), np.float32),
                   xq=np.empty((N_CORES * 1024, 128), np.int8),
                   xs=np.empty((N_CORES * 1024, 1), np.float32))
    _ST = dict(nc=nc, fn=fn, shard=shard, in_names=in_names, out_names=out_names,
               prev=list(outs), wdig=None, devw=None, jax=jax, hostbuf=hostbuf)
    return _ST


_NPC = {}   # input name -> (original object, numpy copy); for jax-array inputs


def _to_numpy_inputs(inputs):
    """Convert possibly-jax-array inputs to numpy, caching by name+identity so
    repeated calls with the same (immutable) jax arrays fetch only once, and
    prefetching all pending conversions async so the d2h rounds overlap."""
    out, pending = {}, []
    for k, v in inputs.items():
        if isinstance(v, np.ndarray):
            out[k] = v
            continue
        ent = _NPC.get(k)
        if ent is not None and ent[0] is v:
            out[k] = ent[1]
            continue
        try:
            v.copy_to_host_async()
        except Exception:
            pass
        pending.append((k, v))
    for k, v in pending:
        a = np.asarray(v)
        _NPC[k] = (v, a)
        out[k] = a
    return out


def _kernel_once(inputs) -> np.ndarray:
    import jax
    inputs = _to_numpy_inputs(inputs)
    st = _get_state()
    x2 = np.ascontiguousarray(np.asarray(inputs["x"], np.float32)
                              ).reshape(N_CORES * 1024, 128)
    buf = st["hostbuf"]
    xqf, xq, xs = buf["xqf"], buf["xq"], buf["xs"]
    np.abs(x2, out=xqf)
    am = xqf.max(axis=1, keepdims=True)
    np.maximum(am, 1e-30, out=am)
    np.divide(127.0, am, out=am)
    np.multiply(x2, am, out=xqf)
    np.rint(xqf, out=xqf)
    np.copyto(xq, xqf, casting='unsafe')          # exact: xqf is integral
    np.divide(1.0, am, out=xs)                    # = absmax/127
    xdev = jax.device_put(xq, st["shard"])        # async
    xsdev = jax.device_put(xs, st["shard"])
    dyn = {"x_tok": xdev, "x_scale": xsdev}

    def _upload_weights():
        g = _prep_weights(inputs)
        st["devw"] = {nm: jax.device_put(g[nm], st["shard"]) for nm in g}

    def _dispatch():
        args = [dyn[nm] if nm in dyn else st["devw"][nm] for nm in st["in_names"]]
        outs = st["fn"](*args, *st["prev"])
        st["prev"] = list(outs)
        for ot in outs:
            ot.copy_to_host_async()               # overlap the two d2h fetches
        return outs

    if st["devw"] is None:                        # first call: must check first
        st["wdig"] = _weight_digest(inputs)
        _upload_weights()
        outs = _dispatch()
    else:
        # optimistic dispatch with cached weights; digest check overlaps exec
        outs = _dispatch()
        dig = _weight_digest(inputs)
        if dig != st["wdig"]:                     # stale: redo with new weights
            st["wdig"] = dig
            _upload_weights()
            outs = _dispatch()
    o = {nm: outs[i] for i, nm in enumerate(st["out_names"])}
    res8 = np.asarray(o["out8"])                      # (8192, 128) int8
    osc = np.asarray(o["out_s"])                      # (8192, 1) f32
    return np.multiply(res8, osc, dtype=np.float32).reshape(B, Dd, H, W, DM)


def _reset_state():
    """Tear down the jax backend + all cached state (recovers from a wedged
    device the same way a process restart does)."""
    global _ST
    _ST = None
    try:
        import jax
        jax.clear_caches()
        from jax._src import xla_bridge as xb
        xb._clear_backends()
    except Exception:
        pass


def kernel(**inputs) -> np.ndarray:
    import time
    for attempt, backoff in enumerate((2.0, 5.0, 10.0, None)):
        try:
            return _kernel_once(inputs)
        except Exception:
            if backoff is None:
                raise
            _reset_state()
            time.sleep(backoff)
